# revision 30
# baseline (speedup 1.0000x reference)
"""Causal self-attention (B=8, T=1024, C=1024, H=16) on 8 trn2 NeuronCores.

Data-parallel over batch: each core computes one batch element's full
attention layer; no collectives. All matmuls in bf16 (fp32 PSUM accum).

Host pre-transposes inputs so every contraction dim lands on partitions:
  xT    [C, T]        x[b].T                      (bf16)
  wqk   [C, 8, 256]   w_attn[:2C].T, k/q packed per head-pair (bf16)
  wvT   [C, C]        w_attn[2C:].T               (bf16)
  wpT   [C, C]        w_proj.T                    (bf16)
  bbias [1, C]        b_proj                      (bf16)
  tri   [128, 128]    causal 0/1 mask, [j, i], 1 when i >= j (bf16)
  id128 [128, 128]    identity (bf16)
  ones1 [1, 128]      ones row (bf16)

Per-core pipeline:
  V-proj  v[t, vj] = xT.T @ wvT, staged as v_aug tiles [t, head, v|1|pad]
  per pair m: K/Q-proj -> kT/qT [feat, t] bf16; the two heads' QK^T
    matmuls are interleaved (even head on PE rows 0:64, odd on 64:128 ->
    concurrent row groups); exp(s/8) -> pT bf16 (no max-subtraction:
    logits are ~N(0, 0.17)); causal mask applied POST-exp as a 0/1
    multiply on the diagonal block only (Vector for head0, GpSimd for
    head1) - keeps the Act engine exp-only;
    AV per head: ya[65, i] += v_aug.T @ pT (row 64 = softmax denom via
    the ones column); ya evicted to yr bf16; denom row scatter-DMA'd
    from PSUM to [128, 8], reciprocal, DRAM-hop broadcast to bc;
    yt = yr * bc (bf16)
  out-proj: bias injected via a K=1 matmul (ones1 x bbias); pass A
    (bias + pairs 0..6, per-(tb,half) psum groups run to completion)
    is split around AV(7): tb 0..4 fill the PE while exp(7) drains,
    tb 5..7 cover the pair-7 softmax-denominator chain latency; each
    group evicts to osA bf16. pass B (pair 7 + osA re-injected via an
    identity matmul) is the only PE tail, with per-512-half evictions
    and stores alternating the sync/scalar DMA queues.
"""
import sys
from contextlib import ExitStack

sys.path.insert(0, "/opt/trn_rl_repo")
import numpy as np
import ml_dtypes

from concourse import bacc, mybir
from concourse import tile
from concourse.bass_utils import run_bass_kernel_spmd

B, T, C = 8, 1024, 1024
H = 16
D = C // H  # 64
NCORES = 8
NPAIR = H // 2  # 8
NTB = T // 128  # 8
NCB = C // 128  # 8
F32 = mybir.dt.float32
BF16 = mybir.dt.bfloat16
AF = mybir.ActivationFunctionType
SCALE = 1.0 / 8.0  # 1/sqrt(D)
BF = ml_dtypes.bfloat16


def build():
    nc = bacc.Bacc(target_bir_lowering=False)
    xT = nc.dram_tensor("xT", [C, T], BF16, kind="ExternalInput")
    wqk = nc.dram_tensor("wqk", [C, NPAIR, 256], BF16, kind="ExternalInput")
    wvT = nc.dram_tensor("wvT", [C, C], BF16, kind="ExternalInput")
    wpT = nc.dram_tensor("wpT", [C, C], BF16, kind="ExternalInput")
    bbias = nc.dram_tensor("bbias", [1, C], BF16, kind="ExternalInput")
    tri = nc.dram_tensor("tri", [128, 128], BF16, kind="ExternalInput")
    id128 = nc.dram_tensor("id128", [128, 128], BF16, kind="ExternalInput")
    ones1 = nc.dram_tensor("ones1", [1, 128], BF16, kind="ExternalInput")
    out = nc.dram_tensor("out", [T, C], F32, kind="ExternalOutput")

    with tile.TileContext(nc) as tc, ExitStack() as top:
        const = top.enter_context(tc.tile_pool(name="const", bufs=1))
        ytp = top.enter_context(tc.tile_pool(name="yt", bufs=1))
        smp = top.enter_context(tc.tile_pool(name="sm", bufs=1))
        osp = top.enter_context(tc.tile_pool(name="os", bufs=1))
        psa = top.enter_context(tc.tile_pool(name="psa", bufs=2, space="PSUM"))
        psb = top.enter_context(tc.tile_pool(name="psb", bufs=2, space="PSUM"))
        dramp = top.enter_context(tc.tile_pool(name="dram", bufs=1, space="DRAM"))

        pstat = const.tile([128, 128], BF16, name="pstat")
        nc.scalar.dma_start(out=pstat[:], in_=xT[0:128, 0:128])
        pmov = const.tile([128, 512], BF16, name="pmov")
        nc.scalar.dma_start(out=pmov[:], in_=xT[0:128, 0:512])
        trit = const.tile([128, 128], BF16, name="trit")
        nc.gpsimd.dma_start(out=trit[:], in_=tri[:])
        idt = const.tile([128, 128], BF16, name="idt")
        nc.gpsimd.dma_start(out=idt[:], in_=id128[:])
        onest = const.tile([1, 128], BF16, name="onest")
        nc.gpsimd.dma_start(out=onest[:], in_=ones1[:])
        bbt = const.tile([1, C], BF16, name="bbt")
        nc.gpsimd.dma_start(out=bbt[:], in_=bbias[:])

        # warm-up primer: keep the PE busy (and the HAM un-throttled)
        # while the initial x/w DMAs land. Results are never read.
        prim = psa.tile([128, 512], F32, name="prim", tag="a", bufs=2)
        for _ in range(14):
            nc.tensor.matmul(prim[:, 0:512], pstat[:], pmov[:],
                             start=True, stop=True)

        yts = []
        osas = {}
        with ExitStack() as mid:
            xp = mid.enter_context(tc.tile_pool(name="xp", bufs=1))
            vtp = mid.enter_context(tc.tile_pool(name="vt", bufs=1))
            wq = mid.enter_context(tc.tile_pool(name="wq", bufs=1))
            wqms = {}

            def load_wqm(mm_):
                wqm_ = wq.tile([128, NCB, 256], BF16, name=f"wqm{mm_}",
                               tag="wqm", bufs=2)
                nc.sync.dma_start(
                    out=wqm_[:],
                    in_=wqk[:, mm_, :].rearrange("(cb p) f -> p cb f",
                                                 p=128))
                wqms[mm_] = wqm_

            xts = []
            for cb in range(NCB):
                xt = xp.tile([128, T], BF16, name=f"x{cb}", tag="x", bufs=NCB)
                xeng = nc.scalar if cb < 4 else nc.gpsimd
                xeng.dma_start(out=xt[:],
                               in_=xT[cb * 128:(cb + 1) * 128, :])
                xts.append(xt)

            # ---- V projection ----
            vts = []
            with tc.tile_pool(name="wv", bufs=1) as wv:
                wvt = wv.tile([128, NCB, C], BF16, name="wvt")
                for cb in range(NCB):
                    weng = nc.sync if cb < 4 else nc.scalar
                    weng.dma_start(
                        out=wvt[:, cb, :],
                        in_=wvT[cb * 128:(cb + 1) * 128, :])
                load_wqm(0)
                load_wqm(1)
                for tb in range(NTB):
                    vt = vtp.tile([128, NPAIR, 132], BF16, name=f"v{tb}",
                                  tag="v", bufs=NTB)
                    ps = psb.tile([128, 1024], F32, name="vps", tag="att",
                                  bufs=2)
                    for half in range(2):
                        for cb in range(NCB):
                            nc.tensor.matmul(
                                ps[:, half * 512:(half + 1) * 512],
                                xts[cb][:, tb * 128:(tb + 1) * 128],
                                wvt[:, cb, half * 512:(half + 1) * 512],
                                start=(cb == 0), stop=(cb == NCB - 1))
                    psv = ps[:, 0:1024].rearrange("p (pr f) -> p pr f",
                                                  f=128)
                    # both evictions on vector: the scalar queue is busy
                    # draining the x-tile DMAs during the V phase
                    nc.vector.tensor_copy(vt[:, :, 0:64], psv[:, :, 0:64])
                    nc.vector.tensor_copy(vt[:, :, 66:130], psv[:, :, 64:128])
                    vt2 = vt.rearrange("p pr (two f) -> p (pr two) f", f=66)
                    nc.gpsimd.memset(vt2[:, :, 64:65], 1.0)
                    vts.append(vt)

            # ---- attention, per head pair ----
            with ExitStack() as att_stack:
                kqp = att_stack.enter_context(tc.tile_pool(name="kq", bufs=1))
                ptp = att_stack.enter_context(tc.tile_pool(name="pt", bufs=1))
                wp = att_stack.enter_context(tc.tile_pool(name="wp", bufs=1))

                wpt = wp.tile([128, NCB, C], BF16, name="wpt")
                nc.gpsimd.dma_start(
                    out=wpt[:],
                    in_=wpT[:].rearrange("(cb p) j -> p cb j", p=128))

                kqs = {}

                def kqproj(mm_):
                    wqm_ = wqms[mm_]
                    km_ = kqp.tile([128, T], BF16, name=f"k{mm_}", tag="km",
                                   bufs=2)
                    qm_ = kqp.tile([128, T], BF16, name=f"q{mm_}", tag="qm",
                                   bufs=2)
                    for kq in range(2):  # k pass, q pass
                        dst = km_ if kq == 0 else qm_
                        for th in range(2):
                            ps = psa.tile([128, 512], F32, name="kqps",
                                          tag="a", bufs=2)
                            for cb in range(NCB):
                                nc.tensor.matmul(
                                    ps[:],
                                    wqm_[:, cb, kq * 128:(kq + 1) * 128],
                                    xts[cb][:, th * 512:(th + 1) * 512],
                                    start=(cb == 0), stop=(cb == NCB - 1))
                            nc.vector.tensor_copy(
                                dst[:, th * 512:(th + 1) * 512], ps[:])
                    kqs[mm_] = (km_, qm_)

                def outproj_passA(tbs):
                    # bias + pairs 0..6 of the output projection, for the
                    # given tb blocks (each psum group runs to completion)
                    for tb in tbs:
                        for half in range(2):
                            g = tb * 2 + half
                            ps = psa.tile([128, 512], F32, name="pps",
                                          tag="a", bufs=2)
                            nc.tensor.matmul(
                                ps[:], onest[:],
                                bbt[:, half * 512:(half + 1) * 512],
                                start=True, stop=False)
                            for m in range(7):
                                nc.tensor.matmul(
                                    ps[:],
                                    yts[m][:, tb * 128:(tb + 1) * 128],
                                    wpt[:, m, half * 512:(half + 1) * 512],
                                    start=False, stop=(m == 6))
                            osa = osp.tile([128, 512], BF16, name=f"osa{g}",
                                           tag="osa", bufs=16)
                            if g % 2 == 0:
                                nc.scalar.copy(osa[:], ps[:])
                            else:
                                nc.vector.tensor_copy(osa[:], ps[:])
                            osas[g] = osa

                def outproj_passB():
                    # pair 7 + re-injected pass-A partials; evict + store
                    # per 512-half, stores alternating sync/scalar queues
                    with tc.tile_pool(name="ost", bufs=1) as ostp:
                        for tb in range(NTB):
                            for half in range(2):
                                g = tb * 2 + half
                                ps = psa.tile([128, 512], F32, name="pps",
                                              tag="a", bufs=2)
                                nc.tensor.matmul(
                                    ps[:],
                                    yts[7][:, tb * 128:(tb + 1) * 128],
                                    wpt[:, 7, half * 512:(half + 1) * 512],
                                    start=True, stop=False)
                                nc.tensor.matmul(
                                    ps[:], idt[:], osas[g][:],
                                    start=False, stop=True)
                                ost = ostp.tile([128, 512], F32, name="ost",
                                                tag="ost", bufs=4)
                                if g % 2 == 0:
                                    nc.scalar.copy(ost[:], ps[:])
                                else:
                                    nc.vector.tensor_copy(ost[:], ps[:])
                                deng = nc.sync if g % 2 == 0 else nc.scalar
                                deng.dma_start(
                                    out=out[tb * 128:(tb + 1) * 128,
                                            half * 512:(half + 1) * 512],
                                    in_=ost[:])

                load_wqm(0)
                load_wqm(1)
                kqproj(0)
                for m in range(NPAIR):
                    km, qm = kqs[m]
                    yt = ytp.tile([128, T], BF16, name=f"yt{m}", tag="yt",
                                  bufs=NPAIR)
                    yts.append(yt)

                    # QK^T interleaved across the two heads: even head in PE
                    # rows 0:64, odd head in rows 64:128 -> concurrent.
                    pts = {0: [], 1: []}
                    for jb in range(NTB):
                        w = T - jb * 128
                        for hp in range(2):
                            p0 = hp * 64
                            pt = ptp.tile([128, w], BF16,
                                          name=f"pt{jb}_{hp}",
                                          tag=f"pt{jb}", bufs=2)
                            pts[hp].append(pt)
                            att = psb.tile([128, 1024], F32, name="att",
                                           tag="att", bufs=2)
                            for ch in range(2):
                                i0 = max(jb * 128, ch * 512)
                                cw = (ch + 1) * 512 - i0
                                if cw <= 0:
                                    continue
                                nc.tensor.matmul(
                                    att[:, i0:i0 + cw],
                                    km[p0:p0 + 64,
                                       jb * 128:(jb + 1) * 128],
                                    qm[p0:p0 + 64, i0:i0 + cw],
                                    start=True, stop=True)
                            nc.scalar.activation(
                                pt[:, 0:w], att[:, jb * 128:T], AF.Exp,
                                scale=SCALE)
                            # causal mask: zero i<j in the diagonal block
                            if hp == 0:
                                nc.vector.tensor_mul(
                                    pt[:, 0:128], pt[:, 0:128], trit[:])
                            else:
                                nc.gpsimd.tensor_mul(
                                    pt[:, 0:128], pt[:, 0:128], trit[:])

                    if m + 1 < NPAIR:
                        kqproj(m + 1)  # PE filler while exps drain
                    if m == NPAIR - 1:
                        outproj_passA(range(0, 5))  # filler for last exps

                    for hp in range(2):  # AV + softmax denom per head
                        voff = 66 * hp
                        ya = [psb.tile([128, 512], F32, name="ya",
                                       tag="ya", bufs=2) for _ in range(2)]
                        for jb in range(NTB):
                            for ch in range(2):
                                if jb * 128 >= (ch + 1) * 512:
                                    continue
                                i0 = max(jb * 128, ch * 512)
                                cw = (ch + 1) * 512 - i0
                                first = (jb == 0)
                                last = (jb == NTB - 1) or \
                                    (ch == 0 and jb == 3)
                                nc.tensor.matmul(
                                    ya[ch][0:65,
                                           i0 - ch * 512:i0 - ch * 512 + cw],
                                    vts[jb][:, m, voff:voff + 65],
                                    pts[hp][jb][:, i0 - jb * 128:
                                                i0 - jb * 128 + cw],
                                    start=first, stop=last)
                        # evict psum: yr rows 0:64 = raw y, row 64 = denom
                        # (bf16); denom row then scatter-DMA'd to [128, 8]
                        yr = smp.tile([65, T], BF16, name="yr", tag="yr",
                                      bufs=3)
                        dtr = smp.tile([128, 8], BF16, name="dtr", tag="dtr",
                                       bufs=2)
                        dd0 = dramp.tile([1, T], BF16, name="dd0", tag="dd0",
                                         bufs=2)
                        for ch in range(2):
                            nc.vector.tensor_copy(
                                yr[0:65, ch * 512:(ch + 1) * 512],
                                ya[ch][0:65, 0:512])
                            nc.sync.dma_start(
                                out=dd0[0:1, ch * 512:(ch + 1) * 512],
                                in_=yr[64:65, ch * 512:(ch + 1) * 512])
                            nc.sync.dma_start(
                                out=dtr[ch * 64:(ch + 1) * 64, :],
                                in_=dd0[0, ch * 512:(ch + 1) * 512]
                                .rearrange("(p q) -> p q", q=8))
                        rtr = smp.tile([128, 8], BF16, name="rtr", tag="rtr",
                                       bufs=2)
                        with nc.allow_low_precision(
                                reason="softmax denom recip in bf16"):
                            nc.vector.reciprocal(rtr[:], dtr[:])
                        dd = dramp.tile([1, T], BF16, name="dd", tag="dd",
                                        bufs=2)
                        nc.gpsimd.dma_start(
                            out=dd[0, :].rearrange("(p q) -> p q", q=8),
                            in_=rtr[:])
                        bc = smp.tile([64, T], BF16, name="bc", tag="bc",
                                      bufs=3)
                        nc.gpsimd.dma_start(
                            out=bc[:], in_=dd[0, :].partition_broadcast(64))
                        # normalize into YT pair tile
                        if hp == 0:
                            nc.vector.tensor_mul(yt[0:64, :], yr[0:64, :],
                                                 bc[:])
                        else:
                            ytmp = smp.tile([64, T], BF16, name="ytmp",
                                            tag="ytmp", bufs=2)
                            nc.vector.tensor_mul(ytmp[:], yr[0:64, :], bc[:])
                            nc.sync.dma_start(out=yt[64:128, :], in_=ytmp[:])
                    if m + 2 < NPAIR:
                        load_wqm(m + 2)
                    if m == NPAIR - 1:
                        # covers the pair-7 softmax-denominator chain latency
                        outproj_passA(range(5, 8))

                # ---- output projection tail ----
                outproj_passB()

    nc.compile()
    return nc


_NC = None


def _get_nc():
    global _NC
    if _NC is None:
        _NC = build()
    return _NC


def prep_inputs(x, w_attn, w_proj, b_proj):
    x = np.asarray(x, dtype=np.float32)
    w_attn = np.asarray(w_attn, dtype=np.float32)
    w_proj = np.asarray(w_proj, dtype=np.float32)
    b_proj = np.asarray(b_proj, dtype=np.float32)
    ki = np.ascontiguousarray(w_attn[0:C].T).reshape(C, NPAIR, 128)
    qi = np.ascontiguousarray(w_attn[C:2 * C].T).reshape(C, NPAIR, 128)
    wqkv = np.ascontiguousarray(
        np.concatenate([ki, qi], axis=2)).astype(BF)
    wvTv = np.ascontiguousarray(w_attn[2 * C:3 * C].T).astype(BF)
    wpTv = np.ascontiguousarray(w_proj.T).astype(BF)
    bbv = b_proj.reshape(1, C).astype(BF)
    ii = np.arange(128)
    trv = (ii[None, :] >= ii[:, None]).astype(BF)
    shared = {"wqk": wqkv, "wvT": wvTv, "wpT": wpTv, "bbias": bbv,
              "tri": trv, "id128": np.eye(128, dtype=BF),
              "ones1": np.ones((1, 128), dtype=BF)}
    in_maps = []
    for b in range(B):
        im = dict(shared)
        im["xT"] = np.ascontiguousarray(x[b].T).astype(BF)
        in_maps.append(im)
    return in_maps


def kernel(x, w_attn, w_proj, b_proj):
    nc = _get_nc()
    in_maps = prep_inputs(x, w_attn, w_proj, b_proj)
    res = run_bass_kernel_spmd(nc, in_maps, core_ids=list(range(NCORES)))
    return np.stack([res.results[b]["out"] for b in range(B)]).astype(np.float32)
